# revision 3
# baseline (speedup 1.0000x reference)
"""AriaGroupedGEMM (MoE grouped GEMM) on 8 TRN2 NeuronCores.

Problem: input [4096, 2048] f32, weight [8, 2048, 2048] f32,
tokens_per_expert [8] int32 (tokens pre-sorted by expert).
out[i] = input[i] @ weight[expert_of(i)].

Strategy: expert-parallel. Core g owns expert g's weight and its token
group. Each core runs a dense [T_pad, 2048] @ [2048, 2048] GEMM in bf16
(fp32 PSUM accumulation). Host pre-swizzles operands into SBUF-native,
block-contiguous layouts so every DMA is one contiguous run per
partition line.

Schedule notes (see trace analysis):
- Warm-up matmuls (each its own accumulation group) start as early as
  possible (wu tiles memset on the otherwise-idle Vector engine) and
  cover the HAM SHORT window so real MMs run at 2.4GHz from the start.
- Input DMAs are split across BOTH HWDGE rings (sync + scalar) with
  byte-balanced prefixes in consumption order: first bytes land ~2x
  sooner and aggregate bandwidth is higher.
- Narrow head blocks (256 cols) + small chunks gate the first real MM
  on ~256KB of data; a (chunk, m) diagonal consumes in arrival order.
- PSUM->SBUF casts run on the Scalar engine (which also issues the
  output DMAs: same-engine FIFO ordering needs no semaphores), batched
  into per-(m, half) staging tiles so there are only ~10 output DMAs.
- Few, large bulk DMAs (1MB) keep the semaphore/tick count low - the
  kernel epilogue serializes one EVENT_SEMAPHORE per outstanding sem
  per engine, so fewer DMAs directly shrink the fixed tail.
"""
import sys
import functools

for _p in ("/opt/trn_rl_repo", "/root/.axon_site/_ro/trn_rl_repo"):
    if _p not in sys.path:
        sys.path.insert(0, _p)

import numpy as np
import ml_dtypes

import concourse.mybir as mybir
import concourse.tile as tile
from concourse import bacc
from concourse import bass_utils

P = 128
K = 2048            # in_features (contraction)
N = 2048            # out_features
G = 8               # experts == cores
KO = K // P         # 16 k-subtiles

COMPUTE_DT = mybir.dt.bfloat16
NP_COMPUTE = ml_dtypes.bfloat16
OUT_DT = mybir.dt.bfloat16      # psum(f32) -> bf16 on the way out; host upcasts

N_WARMUP_MM = 9     # N=512 warm-up matmuls (HAM ramp) before data lands

# column blocks: two narrow head blocks shrink the DMA bytes gating the
# first outputs; the tail blocks run at the efficient 512-wide MM rate.
BLOCKS = [(0, 256), (256, 256), (512, 512), (1024, 512), (1536, 512)]
# w chunking per block: head blocks 4 chunks (128KB), bulk 2 chunks (1MB)
BLOCK_CHUNKS = [4, 2, 2, 2, 2]


@functools.lru_cache(maxsize=4)
def _build(t_pad: int):
    """Build + compile the per-core GEMM graph for token-pad t_pad."""
    mt = t_pad // P  # m tiles of 128 tokens

    nc = bacc.Bacc("TRN2", target_bir_lowering=False, debug=False)

    # host-swizzled SBUF-native layouts (contiguous per partition line):
    # xt[mi, p, ko, j] = X[mi*P + j, ko*P + p]
    # w is block-major: for each block b (c0, width):
    #   wblk_b[p, ko, j] = W[ko*P + p, c0 + j]; blocks concatenated on the
    #   free axis so every (block, ko-range) chunk is ONE contiguous run
    #   per partition line.
    xt_d = nc.dram_tensor(
        "xt", [mt, P, KO, P], COMPUTE_DT, kind="ExternalInput"
    ).ap()
    WTOT = KO * N
    w_d = nc.dram_tensor("w", [P, WTOT], COMPUTE_DT, kind="ExternalInput").ap()
    out_d = nc.dram_tensor("out", [t_pad, N], OUT_DT, kind="ExternalOutput").ap()

    NBK = len(BLOCKS)
    blk_off = []
    off = 0
    for (c0, width) in BLOCKS:
        blk_off.append(off)
        off += KO * width

    diag = mt <= 6

    with tile.TileContext(nc) as tc:
        with (
            tc.tile_pool(name="xt_p", bufs=1) as xt_p,
            tc.tile_pool(name="w_p", bufs=1) as w_p,
            tc.tile_pool(name="st_p", bufs=1) as st_p,
            tc.tile_pool(name="wu_p", bufs=1) as wu_p,
            tc.tile_pool(name="ps", bufs=7, space="PSUM") as ps,
            tc.tile_pool(name="wu_ps_p", bufs=1, space="PSUM") as wu_ps_p,
        ):
            # --- PE warm-up: independent single-MM groups on a scratch
            # psum bank. Tiles are memset on Vector (idle at kernel start)
            # so the first warm-up MM issues as early as possible; they
            # run during the initial DMA latency and lift the HAM clock
            # gate to 2.4GHz before the real stream begins.
            wu_lhs = wu_p.tile([P, P], COMPUTE_DT, tag="wu_lhs")
            wu_rhs = wu_p.tile([P, 512], COMPUTE_DT, tag="wu_rhs")
            nc.vector.memset(wu_lhs[:], 0.0)
            nc.vector.memset(wu_rhs[:], 0.0)
            wu_ps = wu_ps_p.tile([P, 512], mybir.dt.float32, tag="wu_ps")
            for i in range(N_WARMUP_MM):
                nc.tensor.matmul(wu_ps[:], wu_lhs[:], wu_rhs[:],
                                 start=True, stop=True)

            # --- input DMAs: consumption order, byte-balanced across the
            # two HWDGE rings (sync + scalar) so both stream in parallel.
            xt_t = {}           # (mi, half) -> tile  (m0 split in halves)
            w_c = {}            # (b, c) -> tile

            def dma_items():
                # yields (kind, key, bytes) in consumption order
                if diag:
                    # m0's xt in quarters paired with w0 chunks: the
                    # (c, m) diagonal consumes in exact arrival order
                    yield ("xt0", (0, 0), P * 4 * P * 2)
                    yield ("w", (0, 0), P * 4 * 256 * 2)
                    yield ("xt0", (0, 1), P * 4 * P * 2)
                    yield ("w", (0, 1), P * 4 * 256 * 2)
                    yield ("xt", (1, None), P * KO * P * 2)
                    yield ("xt0", (0, 2), P * 4 * P * 2)
                    yield ("w", (0, 2), P * 4 * 256 * 2)
                    yield ("xt0", (0, 3), P * 4 * P * 2)
                    yield ("w", (0, 3), P * 4 * 256 * 2)
                    for mi in range(2, mt):
                        yield ("xt", (mi, None), P * KO * P * 2)
                else:
                    for mi in range(mt):
                        yield ("xt", (mi, None), P * KO * P * 2)
                    for c in range(BLOCK_CHUNKS[0]):
                        yield ("w", (0, c), P * (KO // BLOCK_CHUNKS[0]) * BLOCKS[0][1] * 2)
                for b in range(1, NBK):
                    nch = BLOCK_CHUNKS[b]
                    for c in range(nch):
                        yield ("w", (b, c), P * (KO // nch) * BLOCKS[b][1] * 2)

            qbytes = {0: 0, 1: 0}
            queues = [nc.sync, nc.scalar]
            for kind, key, nbytes in dma_items():
                q = 0 if qbytes[0] <= qbytes[1] else 1
                qbytes[q] += nbytes
                eng = queues[q]
                if kind == "xt":
                    mi = key[0]
                    t = xt_p.tile([P, KO, P], COMPUTE_DT, tag=f"xt_m{mi}",
                                  name=f"xt_m{mi}")
                    eng.dma_start(t[:], xt_d[mi])
                    for q4 in range(4):
                        xt_t[(mi, q4)] = (t, q4 * 4)
                elif kind == "xt0":
                    mi, q4 = key
                    t = xt_p.tile([P, 4, P], COMPUTE_DT, tag=f"xt_m{mi}q{q4}",
                                  name=f"xt_m{mi}q{q4}")
                    eng.dma_start(t[:], xt_d[mi, :, q4 * 4:(q4 + 1) * 4, :])
                    xt_t[(mi, q4)] = (t, 0)
                else:
                    b, c = key
                    c0, width = BLOCKS[b]
                    kch = KO // BLOCK_CHUNKS[b]
                    t = w_p.tile([P, kch * width], COMPUTE_DT,
                                 tag=f"w_b{b}_c{c}", name=f"w_b{b}_c{c}")
                    o0 = blk_off[b] + c * kch * width
                    eng.dma_start(t[:], w_d[:, o0:o0 + kch * width])
                    w_c[(b, c)] = t

            def xt_ap(mi, ko):
                t, base = xt_t[(mi, ko // 4)]
                return t[:, base + (ko % 4), :]

            def w_ap(b, ko):
                kch = KO // BLOCK_CHUNKS[b]
                t = w_c[(b, ko // kch)]
                width = BLOCKS[b][1]
                j = ko % kch
                return t[:, j * width:(j + 1) * width]

            # --- output staging: per (m, half) [P, 1024] bf16 tiles.
            # Scalar does the psum->sbuf casts AND the out DMAs, so the
            # cast -> dma ordering is engine-FIFO (no semaphores).
            if diag:
                st = {(m, h): st_p.tile([P, 1024], OUT_DT, tag=f"st_{m}_{h}",
                                        name=f"st_{m}_{h}")
                      for m in range(mt) for h in range(2)}

            # half-boundaries: blocks 0,1,2 -> half 0 (cols 0:1024),
            # blocks 3,4 -> half 1 (cols 1024:2048)
            BLK_HALF = [(0, 0), (0, 256), (0, 512), (1, 0), (1, 512)]

            def cast_block(b, m, psum_t):
                h, so = BLK_HALF[b]
                width = BLOCKS[b][1]
                nc.scalar.copy(st[(m, h)][:, so:so + width], psum_t[:])

            def emit_out_half(m, h, eng=None):
                (eng or nc.scalar).dma_start(
                    out_d[m * P:(m + 1) * P, h * 1024:(h + 1) * 1024],
                    st[(m, h)][:])

            # --- compute ---
            if diag:
                # head block 0: (chunk, m) diagonal in data-arrival order
                psums0 = {
                    m: ps.tile([P, 256], mybir.dt.float32, tag="psum",
                               name=f"psum_0_{m}")
                    for m in range(mt)
                }
                pairs = sorted(
                    ((c, m) for c in range(4) for m in range(mt)),
                    key=lambda cm: (cm[0] + cm[1], cm[0]),
                )
                for c, m in pairs:
                    for ko in range(c * 4, (c + 1) * 4):
                        nc.tensor.matmul(
                            psums0[m][:], xt_ap(m, ko), w_ap(0, ko),
                            start=(ko == 0), stop=(ko == KO - 1),
                        )
                    if c == 3:
                        cast_block(0, m, psums0[m])

                # blocks 1..4: m-major, k-inner (dense same-bank
                # accumulation keeps the PE at the warm back-to-back rate)
                for b in range(1, NBK):
                    width = BLOCKS[b][1]
                    for m in range(mt):
                        last = b == NBK - 1 and m == mt - 1
                        psum_t = ps.tile([P, width], mybir.dt.float32,
                                         tag="psum", name=f"psum_{b}_{m}")
                        for k in range(KO):
                            nc.tensor.matmul(
                                psum_t[:], xt_ap(m, k), w_ap(b, k),
                                start=(k == 0), stop=(k == KO - 1),
                            )
                        if last:
                            # split the final cast+DMA so the tail is short;
                            # last two small DMAs go on both rings in parallel
                            h, so = BLK_HALF[b]
                            nc.scalar.copy(
                                st[(m, h)][:, so:so + 256], psum_t[:, 0:256])
                            nc.scalar.dma_start(
                                out_d[m * P:(m + 1) * P, 1024:1792],
                                st[(m, h)][:, 0:768])
                            nc.scalar.copy(
                                st[(m, h)][:, so + 256:so + 512],
                                psum_t[:, 256:512])
                            nc.sync.dma_start(
                                out_d[m * P:(m + 1) * P, 1792:2048],
                                st[(m, h)][:, 768:1024])
                        else:
                            cast_block(b, m, psum_t)
                            if b == 2:
                                emit_out_half(m, 0)
                            elif b == 4:
                                emit_out_half(m, 1)
            else:
                # generic fallback (mt > 6): m-major over all blocks,
                # per-block cast + direct out DMA
                for b in range(NBK):
                    c0, width = BLOCKS[b]
                    for m in range(mt):
                        psum_t = ps.tile([P, width], mybir.dt.float32,
                                         tag="psum", name=f"psum_{b}_{m}")
                        for k in range(KO):
                            nc.tensor.matmul(
                                psum_t[:], xt_ap(m, k), w_ap(b, k),
                                start=(k == 0), stop=(k == KO - 1),
                            )
                        o_sb = st_p.tile([P, width], OUT_DT,
                                         tag=f"o{(b * mt + m) % 4}",
                                         name=f"o_{b}_{m}")
                        nc.scalar.copy(o_sb[:], psum_t[:])
                        nc.scalar.dma_start(
                            out_d[m * P:(m + 1) * P, c0:c0 + width], o_sb[:])

    nc.compile()
    return nc


def _swizzle_x(x_pad: np.ndarray, t_pad: int) -> np.ndarray:
    # [t_pad, K] f32 -> [mt, P, KO, P] bf16, xt[mi,p,ko,j] = X[mi*P+j, ko*P+p]
    mt = t_pad // P
    v = x_pad.reshape(mt, P, KO, P).transpose(0, 3, 2, 1)
    return np.ascontiguousarray(v.astype(NP_COMPUTE))


def _swizzle_w(w_g: np.ndarray) -> np.ndarray:
    # [K, N] f32 -> [P, KO*N] block-major:
    # for block (c0,width): wblk[p, ko, j] = W[ko*P + p, c0 + j]
    v = w_g.reshape(KO, P, N).transpose(1, 0, 2)  # [P, KO, N]
    parts = [
        np.ascontiguousarray(v[:, :, c0:c0 + width]).reshape(P, KO * width)
        for (c0, width) in BLOCKS
    ]
    return np.ascontiguousarray(np.concatenate(parts, axis=1).astype(NP_COMPUTE))


def _run(input, weight, tokens_per_expert, trace=False, **trace_kwargs):
    inp = np.ascontiguousarray(np.asarray(input), dtype=np.float32)
    wgt = np.ascontiguousarray(np.asarray(weight), dtype=np.float32)
    counts = np.asarray(tokens_per_expert).astype(np.int64)
    num_tokens, k = inp.shape
    assert k == K and wgt.shape == (G, K, N)
    # token group boundaries (matches searchsorted(cumsum, arange, 'right')),
    # clamped to the token range for safety on degenerate counts
    ends = np.minimum(np.cumsum(counts), num_tokens)
    starts = np.minimum(ends - counts, num_tokens)
    sizes = np.maximum(ends - starts, 0)

    t_pad = max(P, int(-(-max(int(sizes.max()), 1) // P)) * P)
    nc = _build(t_pad)

    in_maps = []
    for g in range(G):
        x_pad = np.zeros((t_pad, K), dtype=np.float32)
        x_pad[: sizes[g]] = inp[starts[g]:ends[g]]
        in_maps.append({"xt": _swizzle_x(x_pad, t_pad), "w": _swizzle_w(wgt[g])})

    res = bass_utils.run_bass_kernel_spmd(
        nc, in_maps, core_ids=list(range(G)), trace=trace, **trace_kwargs
    )

    # tokens not covered by any expert group get zero output (matches the
    # reference's masked accumulation)
    out = np.zeros((num_tokens, N), dtype=np.float32)
    for g in range(G):
        out[starts[g]:ends[g]] = res.results[g]["out"][: sizes[g]].astype(np.float32)
    return out, res


def kernel(input, weight, tokens_per_expert):
    out, _ = _run(input, weight, tokens_per_expert)
    return out


# revision 4
# speedup vs baseline: 1.0039x; 1.0039x over previous
"""AriaGroupedGEMM (MoE grouped GEMM) on 8 TRN2 NeuronCores.

Problem: input [4096, 2048] f32, weight [8, 2048, 2048] f32,
tokens_per_expert [8] int32 (tokens pre-sorted by expert).
out[i] = input[i] @ weight[expert_of(i)].

Strategy: expert-parallel. Core g owns expert g's weight and its token
group. Each core runs a dense [T_pad, 2048] @ [2048, 2048] GEMM in bf16
(fp32 PSUM accumulation). Host pre-swizzles operands into SBUF-native,
block-contiguous layouts so every DMA is one contiguous run per
partition line.

Schedule notes (see trace analysis):
- Warm-up matmuls (each its own accumulation group) start as early as
  possible (wu tiles memset on the otherwise-idle Vector engine) and
  cover the HAM SHORT window so real MMs run at 2.4GHz from the start.
- Input DMAs are split across BOTH HWDGE rings (sync + scalar) with
  byte-balanced prefixes in consumption order: first bytes land ~2x
  sooner and aggregate bandwidth is higher.
- Narrow head blocks (256 cols) + small chunks gate the first real MM
  on ~256KB of data; a (chunk, m) diagonal consumes in arrival order.
- PSUM->SBUF casts run on the Scalar engine (which also issues the
  output DMAs: same-engine FIFO ordering needs no semaphores), batched
  into per-(m, half) staging tiles so there are only ~10 output DMAs.
- Few, large bulk DMAs (1MB) keep the semaphore/tick count low - the
  kernel epilogue serializes one EVENT_SEMAPHORE per outstanding sem
  per engine, so fewer DMAs directly shrink the fixed tail.
"""
import sys
import functools

for _p in ("/opt/trn_rl_repo", "/root/.axon_site/_ro/trn_rl_repo"):
    if _p not in sys.path:
        sys.path.insert(0, _p)

import numpy as np
import ml_dtypes

import concourse.mybir as mybir
import concourse.tile as tile
from concourse import bacc
from concourse import bass_utils

P = 128
K = 2048            # in_features (contraction)
N = 2048            # out_features
G = 8               # experts == cores
KO = K // P         # 16 k-subtiles

COMPUTE_DT = mybir.dt.bfloat16
NP_COMPUTE = ml_dtypes.bfloat16
OUT_DT = mybir.dt.bfloat16      # psum(f32) -> bf16 on the way out; host upcasts

N_WARMUP_MM = 5     # N=512 warm-up matmuls (HAM ramp) before data lands

# column blocks: two narrow head blocks shrink the DMA bytes gating the
# first outputs; the tail blocks run at the efficient 512-wide MM rate.
BLOCKS = [(0, 256), (256, 256), (512, 512), (1024, 512), (1536, 512)]
# w chunking per block: head blocks 4 chunks (128KB), bulk 2 chunks (1MB)
BLOCK_CHUNKS = [4, 4, 2, 2, 2]


@functools.lru_cache(maxsize=4)
def _build(t_pad: int):
    """Build + compile the per-core GEMM graph for token-pad t_pad."""
    mt = t_pad // P  # m tiles of 128 tokens

    nc = bacc.Bacc("TRN2", target_bir_lowering=False, debug=False)

    # host-swizzled SBUF-native layouts (contiguous per partition line):
    # xt[mi, p, ko, j] = X[mi*P + j, ko*P + p]
    # w is block-major: for each block b (c0, width):
    #   wblk_b[p, ko, j] = W[ko*P + p, c0 + j]; blocks concatenated on the
    #   free axis so every (block, ko-range) chunk is ONE contiguous run
    #   per partition line.
    xt_d = nc.dram_tensor(
        "xt", [mt, P, KO, P], COMPUTE_DT, kind="ExternalInput"
    ).ap()
    WTOT = KO * N
    w_d = nc.dram_tensor("w", [P, WTOT], COMPUTE_DT, kind="ExternalInput").ap()
    out_d = nc.dram_tensor("out", [t_pad, N], OUT_DT, kind="ExternalOutput").ap()

    NBK = len(BLOCKS)
    blk_off = []
    off = 0
    for (c0, width) in BLOCKS:
        blk_off.append(off)
        off += KO * width

    diag = mt <= 6

    with tile.TileContext(nc) as tc:
        with (
            tc.tile_pool(name="xt_p", bufs=1) as xt_p,
            tc.tile_pool(name="w_p", bufs=1) as w_p,
            tc.tile_pool(name="st_p", bufs=1) as st_p,
            tc.tile_pool(name="wu_p", bufs=1) as wu_p,
            tc.tile_pool(name="ps", bufs=7, space="PSUM") as ps,
            tc.tile_pool(name="wu_ps_p", bufs=1, space="PSUM") as wu_ps_p,
        ):
            # --- PE warm-up: independent single-MM groups on a scratch
            # psum bank. Tiles are memset on Vector (idle at kernel start)
            # so the first warm-up MM issues as early as possible; they
            # run during the initial DMA latency and lift the HAM clock
            # gate to 2.4GHz before the real stream begins.
            wu_lhs = wu_p.tile([P, P], COMPUTE_DT, tag="wu_lhs")
            wu_rhs = wu_p.tile([P, 512], COMPUTE_DT, tag="wu_rhs")
            nc.vector.memset(wu_lhs[:], 0.0)
            nc.vector.memset(wu_rhs[:], 0.0)
            wu_ps = wu_ps_p.tile([P, 512], mybir.dt.float32, tag="wu_ps")
            for i in range(N_WARMUP_MM):
                nc.tensor.matmul(wu_ps[:], wu_lhs[:], wu_rhs[:],
                                 start=True, stop=True)

            # --- input DMAs: consumption order, byte-balanced across the
            # two HWDGE rings (sync + scalar) so both stream in parallel.
            xt_t = {}           # (mi, half) -> tile  (m0 split in halves)
            w_c = {}            # (b, c) -> tile

            def dma_items():
                # yields (kind, key) in exact first-need order of the
                # compute schedule (the per-engine DMA rings are FIFO, so
                # emission order IS arrival order per ring)
                if diag:
                    # head: the block-0 (chunk, m) diagonal first touches
                    # tiles in this order (xt0 quartered, xt1..3 halved)
                    yield ("xt0", (0, 0))      # cell (0,0)
                    yield ("w", (0, 0))
                    yield ("xth", (1, 0))      # cell (0,1)
                    yield ("xt0", (0, 1))      # cell (1,0)
                    yield ("w", (0, 1))
                    if mt > 2:
                        yield ("xth", (2, 0))  # cell (0,2)
                    yield ("xt0", (0, 2))      # cell (2,0)
                    yield ("w", (0, 2))
                    if mt > 3:
                        yield ("xth", (3, 0))  # cell (0,3)
                    yield ("xth", (1, 1))      # cell (2,1)
                    yield ("xt0", (0, 3))      # cell (3,0)
                    yield ("w", (0, 3))
                    if mt > 2:
                        yield ("xth", (2, 1))  # cell (2,2)
                    if mt > 3:
                        yield ("xth", (3, 1))  # cell (2,3)
                    for mi in range(4, mt):
                        yield ("xth", (mi, 0))
                        yield ("xth", (mi, 1))
                else:
                    for mi in range(mt):
                        yield ("xth", (mi, 0))
                        yield ("xth", (mi, 1))
                    for c in range(BLOCK_CHUNKS[0]):
                        yield ("w", (0, c))
                for b in range(1, NBK):
                    for c in range(BLOCK_CHUNKS[b]):
                        yield ("w", (b, c))

            # first two tiles ride the SWDGE (gpsimd) ring: ~1us first
            # byte beats the HWDGE rings' issue+latency, so the first
            # matmul starts sooner. Everything else alternates between
            # the two HWDGE rings in need order.
            queues = [nc.sync, nc.scalar]
            qi = 0
            for idx, (kind, key) in enumerate(dma_items()):
                if idx < 2:
                    eng = nc.gpsimd
                else:
                    eng = queues[qi]
                    qi ^= 1
                if kind == "xth":
                    mi, h = key
                    t = xt_p.tile([P, 8, P], COMPUTE_DT, tag=f"xt_m{mi}h{h}",
                                  name=f"xt_m{mi}h{h}")
                    eng.dma_start(t[:], xt_d[mi, :, h * 8:(h + 1) * 8, :])
                    xt_t[(mi, h * 2)] = (t, 0)
                    xt_t[(mi, h * 2 + 1)] = (t, 4)
                elif kind == "xt0":
                    mi, q4 = key
                    t = xt_p.tile([P, 4, P], COMPUTE_DT, tag=f"xt_m{mi}q{q4}",
                                  name=f"xt_m{mi}q{q4}")
                    eng.dma_start(t[:], xt_d[mi, :, q4 * 4:(q4 + 1) * 4, :])
                    xt_t[(mi, q4)] = (t, 0)
                else:
                    b, c = key
                    c0, width = BLOCKS[b]
                    kch = KO // BLOCK_CHUNKS[b]
                    t = w_p.tile([P, kch * width], COMPUTE_DT,
                                 tag=f"w_b{b}_c{c}", name=f"w_b{b}_c{c}")
                    o0 = blk_off[b] + c * kch * width
                    eng.dma_start(t[:], w_d[:, o0:o0 + kch * width])
                    w_c[(b, c)] = t

            def xt_ap(mi, ko):
                t, base = xt_t[(mi, ko // 4)]
                return t[:, base + (ko % 4), :]

            def w_ap(b, ko):
                kch = KO // BLOCK_CHUNKS[b]
                t = w_c[(b, ko // kch)]
                width = BLOCKS[b][1]
                j = ko % kch
                return t[:, j * width:(j + 1) * width]

            # --- output staging: per (m, half) [P, 1024] bf16 tiles.
            # Scalar does the psum->sbuf casts AND the out DMAs, so the
            # cast -> dma ordering is engine-FIFO (no semaphores).
            if diag:
                st = {(m, h): st_p.tile([P, 1024], OUT_DT, tag=f"st_{m}_{h}",
                                        name=f"st_{m}_{h}")
                      for m in range(mt) for h in range(2)}

            # half-boundaries: blocks 0,1,2 -> half 0 (cols 0:1024),
            # blocks 3,4 -> half 1 (cols 1024:2048)
            BLK_HALF = [(0, 0), (0, 256), (0, 512), (1, 0), (1, 512)]

            def cast_block(b, m, psum_t):
                h, so = BLK_HALF[b]
                width = BLOCKS[b][1]
                nc.scalar.copy(st[(m, h)][:, so:so + width], psum_t[:])

            def emit_out_half(m, h, eng=None):
                (eng or nc.scalar).dma_start(
                    out_d[m * P:(m + 1) * P, h * 1024:(h + 1) * 1024],
                    st[(m, h)][:])

            # --- compute ---
            if diag:
                # head block 0: (chunk, m) diagonal in data-arrival order
                psums0 = {
                    m: ps.tile([P, 256], mybir.dt.float32, tag="psum",
                               name=f"psum_0_{m}")
                    for m in range(mt)
                }
                pairs = sorted(
                    ((c, m) for c in range(4) for m in range(mt)),
                    key=lambda cm: (cm[0] + cm[1], cm[0]),
                )
                for c, m in pairs:
                    for ko in range(c * 4, (c + 1) * 4):
                        nc.tensor.matmul(
                            psums0[m][:], xt_ap(m, ko), w_ap(0, ko),
                            start=(ko == 0), stop=(ko == KO - 1),
                        )
                    if c == 3:
                        cast_block(0, m, psums0[m])

                # blocks 1..4: m-major, k-inner (dense same-bank
                # accumulation keeps the PE at the warm back-to-back rate)
                for b in range(1, NBK):
                    width = BLOCKS[b][1]
                    for m in range(mt):
                        last = b == NBK - 1 and m == mt - 1
                        psum_t = ps.tile([P, width], mybir.dt.float32,
                                         tag="psum", name=f"psum_{b}_{m}")
                        for k in range(KO):
                            nc.tensor.matmul(
                                psum_t[:], xt_ap(m, k), w_ap(b, k),
                                start=(k == 0), stop=(k == KO - 1),
                            )
                        if last:
                            # split the final cast+DMA so the tail is short;
                            # last two small DMAs go on both rings in parallel
                            h, so = BLK_HALF[b]
                            nc.scalar.copy(
                                st[(m, h)][:, so:so + 256], psum_t[:, 0:256])
                            nc.scalar.dma_start(
                                out_d[m * P:(m + 1) * P, 1024:1792],
                                st[(m, h)][:, 0:768])
                            nc.scalar.copy(
                                st[(m, h)][:, so + 256:so + 512],
                                psum_t[:, 256:512])
                            nc.sync.dma_start(
                                out_d[m * P:(m + 1) * P, 1792:2048],
                                st[(m, h)][:, 768:1024])
                        else:
                            cast_block(b, m, psum_t)
                            if b == 2:
                                emit_out_half(m, 0)
                            elif b == 4:
                                emit_out_half(m, 1)
            else:
                # generic fallback (mt > 6): m-major over all blocks,
                # per-block cast + direct out DMA
                for b in range(NBK):
                    c0, width = BLOCKS[b]
                    for m in range(mt):
                        psum_t = ps.tile([P, width], mybir.dt.float32,
                                         tag="psum", name=f"psum_{b}_{m}")
                        for k in range(KO):
                            nc.tensor.matmul(
                                psum_t[:], xt_ap(m, k), w_ap(b, k),
                                start=(k == 0), stop=(k == KO - 1),
                            )
                        o_sb = st_p.tile([P, width], OUT_DT,
                                         tag=f"o{(b * mt + m) % 4}",
                                         name=f"o_{b}_{m}")
                        nc.scalar.copy(o_sb[:], psum_t[:])
                        nc.scalar.dma_start(
                            out_d[m * P:(m + 1) * P, c0:c0 + width], o_sb[:])

    nc.compile()
    return nc


def _swizzle_x(x_pad: np.ndarray, t_pad: int) -> np.ndarray:
    # [t_pad, K] f32 -> [mt, P, KO, P] bf16, xt[mi,p,ko,j] = X[mi*P+j, ko*P+p]
    mt = t_pad // P
    v = x_pad.reshape(mt, P, KO, P).transpose(0, 3, 2, 1)
    return np.ascontiguousarray(v.astype(NP_COMPUTE))


def _swizzle_w(w_g: np.ndarray) -> np.ndarray:
    # [K, N] f32 -> [P, KO*N] block-major:
    # for block (c0,width): wblk[p, ko, j] = W[ko*P + p, c0 + j]
    v = w_g.reshape(KO, P, N).transpose(1, 0, 2)  # [P, KO, N]
    parts = [
        np.ascontiguousarray(v[:, :, c0:c0 + width]).reshape(P, KO * width)
        for (c0, width) in BLOCKS
    ]
    return np.ascontiguousarray(np.concatenate(parts, axis=1).astype(NP_COMPUTE))


def _run(input, weight, tokens_per_expert, trace=False, **trace_kwargs):
    inp = np.ascontiguousarray(np.asarray(input), dtype=np.float32)
    wgt = np.ascontiguousarray(np.asarray(weight), dtype=np.float32)
    counts = np.asarray(tokens_per_expert).astype(np.int64)
    num_tokens, k = inp.shape
    assert k == K and wgt.shape == (G, K, N)
    # token group boundaries (matches searchsorted(cumsum, arange, 'right')),
    # clamped to the token range for safety on degenerate counts
    ends = np.minimum(np.cumsum(counts), num_tokens)
    starts = np.minimum(ends - counts, num_tokens)
    sizes = np.maximum(ends - starts, 0)

    t_pad = max(P, int(-(-max(int(sizes.max()), 1) // P)) * P)
    nc = _build(t_pad)

    in_maps = []
    for g in range(G):
        x_pad = np.zeros((t_pad, K), dtype=np.float32)
        x_pad[: sizes[g]] = inp[starts[g]:ends[g]]
        in_maps.append({"xt": _swizzle_x(x_pad, t_pad), "w": _swizzle_w(wgt[g])})

    res = bass_utils.run_bass_kernel_spmd(
        nc, in_maps, core_ids=list(range(G)), trace=trace, **trace_kwargs
    )

    # tokens not covered by any expert group get zero output (matches the
    # reference's masked accumulation)
    out = np.zeros((num_tokens, N), dtype=np.float32)
    for g in range(G):
        out[starts[g]:ends[g]] = res.results[g]["out"][: sizes[g]].astype(np.float32)
    return out, res


def kernel(input, weight, tokens_per_expert):
    out, _ = _run(input, weight, tokens_per_expert)
    return out


# revision 17
# speedup vs baseline: 1.2577x; 1.2529x over previous
"""AriaGroupedGEMM (MoE grouped GEMM) on 8 TRN2 NeuronCores.

Problem: input [4096, 2048] f32, weight [8, 2048, 2048] f32,
tokens_per_expert [8] int32 (tokens pre-sorted by expert).
out[i] = input[i] @ weight[expert_of(i)].

Strategy: expert-parallel. Core g owns expert g's weight and its token
group. Each core runs a dense [T_pad, 2048] @ [2048, 2048] GEMM in bf16
(fp32 PSUM accumulation). Host pre-swizzles operands into SBUF-native
layouts so every DMA is contiguous per partition line.

Schedule (trace-driven):
- The kernel head is input-bandwidth-bound: the PE can only work on what
  has arrived (~0.29 B/ns/core with all 8 cores streaming). To maximize
  work per fresh byte, phase A covers the 1024-column left half for ALL
  m-tiles at once (x-tiles amortize across the full 1024 width): 13.65us
  of matmuls per 4MB of data -> compute-bound, not supply-bound. Phase A
  uses all 8 PSUM banks (4 m-tiles x 2 banks). Phase B (right half) is
  pure streaming at the N=512 roofline.
- A (k-chunk, m) interleave in phase A consumes tiles in exact arrival
  order; DMAs are emitted in first-need order (per-ring FIFO => arrival
  order); the first items alternate both HWDGE rings, the bulk rides
  sync so the scalar engine stays free for casts + output DMA issues.
- Warm-up matmuls (own accumulation groups, on a phase-A psum bank)
  bridge engine-ready (~7.6us) to first-data (~11us) so the HAM clock
  gate is at 2.4GHz when real matmuls start and never re-throttles.
- PSUM->SBUF casts run on the Scalar engine; outputs are batched per
  (m, half) into [P,1024] staging tiles (few output DMAs); the final
  tile is computed as two 256-wide groups so only a small DMA trails
  the last matmul.
"""
import sys
import functools

for _p in ("/opt/trn_rl_repo", "/root/.axon_site/_ro/trn_rl_repo"):
    if _p not in sys.path:
        sys.path.insert(0, _p)

import numpy as np
import ml_dtypes

import concourse.mybir as mybir
import concourse.tile as tile
from concourse import bacc
from concourse import bass_utils

P = 128
K = 2048            # in_features (contraction)
N = 2048            # out_features
G = 8               # experts == cores
KO = K // P         # 16 k-subtiles
HALF = N // 2       # 1024: phase-A column width

COMPUTE_DT = mybir.dt.bfloat16
NP_COMPUTE = ml_dtypes.bfloat16
OUT_DT = mybir.dt.bfloat16      # psum(f32) -> bf16 on the way out; host upcasts

N_WARMUP_MM = 16    # N=256 warm-up matmuls sized to bridge engine-ready
                    # (~7.6us) to first-data (~11us) in either clock state

# DRAM w layout offsets (elements per partition line):
#   region A: [KO, 1024] k-major (cols 0:1024)          offset 0
#   region B3: [KO, 512] (cols 1024:1536)               offset KO*1024
#   region B4: [KO, 512] (cols 1536:2048)               offset KO*1536
OFF_A = 0
OFF_B3 = KO * HALF
OFF_B4 = KO * (HALF + 512)
WTOT = KO * N

@functools.lru_cache(maxsize=4)
def _build(t_pad: int):
    """Build + compile the per-core GEMM graph for token-pad t_pad."""
    mt = t_pad // P  # m tiles of 128 tokens

    nc = bacc.Bacc("TRN2", target_bir_lowering=False, debug=False)

    # xt[mi, p, ko, j] = X[mi*P + j, ko*P + p]
    xt_d = nc.dram_tensor(
        "xt", [mt, P, KO, P], COMPUTE_DT, kind="ExternalInput"
    ).ap()
    w_d = nc.dram_tensor("w", [P, WTOT], COMPUTE_DT, kind="ExternalInput").ap()
    out_d = nc.dram_tensor("out", [t_pad, N], OUT_DT, kind="ExternalOutput").ap()

    fast = mt <= 4  # phase-A needs 2 psum banks per m-tile

    with tile.TileContext(nc) as tc:
        with (
            tc.tile_pool(name="xt_p", bufs=1) as xt_p,
            tc.tile_pool(name="w_p", bufs=1) as w_p,
            tc.tile_pool(name="st_p", bufs=1) as st_p,
            tc.tile_pool(name="wu_p", bufs=1) as wu_p,
            tc.tile_pool(name="ps", bufs=8, space="PSUM") as ps,
        ):
            # phase-A psum tiles, allocated in the order their banks are
            # freed (casts fire per-m after its ko=15 matmuls) so phase-B's
            # pool cycling lines up with the frees
            psA = {}
            if fast:
                for m in range(mt):
                    for h in range(2):
                        psA[(m, h)] = ps.tile([P, 512], mybir.dt.float32,
                                              tag="psum", name=f"psA_{m}_{h}")

            # --- PE warm-up: independent single-MM groups. The wu tile is
            # memset on Vector (idle at kernel start); the target bank is a
            # phase-A psum tile (the first real group start=True clears it).
            wu = wu_p.tile([P, 256], COMPUTE_DT, tag="wu")
            nc.vector.memset(wu[:], 0.0)
            if fast:
                wu_ps = psA[(0, 0)]
            else:
                wu_ps = ps.tile([P, 256], mybir.dt.float32, tag="psum",
                                name="wu_ps")
            for i in range(N_WARMUP_MM):
                nc.tensor.matmul(wu_ps[:, 0:256], wu[:, 0:P], wu[:],
                                 start=True, stop=True, skip_group_check=True)

            # --- input DMAs in exact first-need order ---
            xt_t = {}           # (mi, quarter) -> (tile, base)
            wA_c = {}           # chunk -> tile [P, 2, 1024]
            wB_c = {}           # (b, c) -> tile [P, 8, 512]

            def dma_items():
                # wA chunk map: chunks 0,1 cover ko 0 and 1 ([P,1,1024],
                # small gating); chunks 2..8 cover ko-pairs ([P,2,1024])
                if fast:
                    # first-need order for ko-outer m-inner rounds:
                    # round ko needs xt[m] quarter ko//4 and wA ko-chunk
                    yield ("xt0", (0, 0))
                    yield ("wA", 0)
                    if mt > 1:
                        yield ("xth", (1, 0))
                    if mt > 2:
                        yield ("xth", (2, 0))
                    if mt > 3:
                        yield ("xth", (3, 0))
                    yield ("wA", 1)
                    yield ("wA", 2)
                    yield ("xt0", (0, 1))
                    yield ("wA", 3)
                    yield ("wA", 4)
                    yield ("xt0", (0, 2))
                    if mt > 1:
                        yield ("xth", (1, 1))
                    if mt > 2:
                        yield ("xth", (2, 1))
                    if mt > 3:
                        yield ("xth", (3, 1))
                    yield ("wA", 5)
                    yield ("wA", 6)
                    yield ("xt0", (0, 3))
                    yield ("wA", 7)
                    yield ("wA", 8)
                else:
                    for mi in range(mt):
                        yield ("xth", (mi, 0))
                        yield ("xth", (mi, 1))
                    for c in range(9):
                        yield ("wA", c)
                for b in (3, 4):
                    for c in range(2):
                        yield ("wB", (b, c))

            # head items alternate the two HWDGE rings (parallel first
            # arrivals); the bulk rides sync (FIFO preserves need order,
            # scalar engine stays free for casts + output DMAs)
            queues = [nc.sync, nc.scalar]
            qi = 0
            for idx, (kind, key) in enumerate(dma_items()):
                if idx < 8:
                    eng = queues[qi]
                    qi ^= 1
                else:
                    eng = nc.sync
                if kind == "xth":
                    mi, h = key
                    t = xt_p.tile([P, 8, P], COMPUTE_DT, tag=f"xt_m{mi}h{h}",
                                  name=f"xt_m{mi}h{h}")
                    eng.dma_start(t[:], xt_d[mi, :, h * 8:(h + 1) * 8, :])
                    xt_t[(mi, h * 2)] = (t, 0)
                    xt_t[(mi, h * 2 + 1)] = (t, 4)
                elif kind == "xt0":
                    mi, q4 = key
                    t = xt_p.tile([P, 4, P], COMPUTE_DT, tag=f"xt_m{mi}q{q4}",
                                  name=f"xt_m{mi}q{q4}")
                    eng.dma_start(t[:], xt_d[mi, :, q4 * 4:(q4 + 1) * 4, :])
                    xt_t[(mi, q4)] = (t, 0)
                elif kind == "wA":
                    c = key
                    nk = 1 if c < 2 else 2
                    ko0 = c if c < 2 else 2 * c - 2
                    t = w_p.tile([P, nk, HALF], COMPUTE_DT, tag=f"wA_{c}",
                                 name=f"wA_{c}")
                    o0 = OFF_A + ko0 * HALF
                    eng.dma_start(t[:], w_d[:, o0:o0 + nk * HALF])
                    wA_c[c] = t
                else:
                    b, c = key
                    off = OFF_B3 if b == 3 else OFF_B4
                    t = w_p.tile([P, 8, 512], COMPUTE_DT, tag=f"wB{b}_{c}",
                                 name=f"wB{b}_{c}")
                    o0 = off + c * 8 * 512
                    eng.dma_start(t[:], w_d[:, o0:o0 + 8 * 512])
                    wB_c[(b, c)] = t

            def xt_ap(mi, ko):
                t, base = xt_t[(mi, ko // 4)]
                return t[:, base + (ko % 4), :]

            def wA_ap(ko, h):
                if ko < 2:
                    c, j = ko, 0
                else:
                    c, j = (ko + 2) // 2, ko % 2
                return wA_c[c][:, j, h * 512:(h + 1) * 512]

            def wB_ap(b, ko, j0=0, w=512):
                return wB_c[(b, ko // 8)][:, ko % 8, j0:j0 + w]

            if fast:
                # output staging: per (m, half) [P, 1024] bf16; Scalar does
                # the casts AND the output DMA issues (same-engine FIFO)
                st = {(m, h): st_p.tile([P, HALF], OUT_DT, tag=f"st_{m}_{h}",
                                        name=f"st_{m}_{h}")
                      for m in range(mt) for h in range(2)}
                # --- phase A: left 1024 columns, ko-outer m-inner rounds.
                # Each round consumes only 256KB of fresh weight per 1.7us
                # of matmuls, so after the first rounds the phase is
                # compute-bound. Per-m casts fire right after that m's
                # ko=15 matmuls, overlapping the rest of the last round.
                for ko in range(KO):
                    for m in range(mt):
                        lhsT = xt_ap(m, ko)
                        for h in range(2):
                            nc.tensor.matmul(
                                psA[(m, h)][:], lhsT, wA_ap(ko, h),
                                start=(ko == 0), stop=(ko == KO - 1),
                            )
                        if ko == KO - 1:
                            nc.scalar.copy(st[(m, 0)][:, 0:512],
                                           psA[(m, 0)][:])
                            nc.scalar.copy(st[(m, 0)][:, 512:1024],
                                           psA[(m, 1)][:])
                            nc.scalar.dma_start(
                                out_d[m * P:(m + 1) * P, 0:HALF],
                                st[(m, 0)][:])

                # --- phase B: right 1024 columns, m-major per 512-block ---
                for b in (3, 4):
                    c0 = HALF if b == 3 else HALF + 512
                    so = 0 if b == 3 else 512
                    for m in range(mt):
                        last = b == 4 and m == mt - 1
                        if last:
                            # two 256-wide groups: group A's cast+DMA
                            # overlaps group B's matmuls -> short tail
                            nc.scalar.dma_start(
                                out_d[m * P:(m + 1) * P, HALF:HALF + 512],
                                st[(m, 1)][:, 0:512])
                            for g in range(2):
                                pg = ps.tile([P, 256], mybir.dt.float32,
                                             tag="psum", name=f"psum_l{g}")
                                for k in range(KO):
                                    nc.tensor.matmul(
                                        pg[:], xt_ap(m, k),
                                        wB_ap(b, k, g * 256, 256),
                                        start=(k == 0), stop=(k == KO - 1),
                                    )
                                nc.scalar.copy(
                                    st[(m, 1)][:, 512 + g * 256:
                                       768 + g * 256], pg[:])
                                eng = nc.scalar if g == 0 else nc.sync
                                eng.dma_start(
                                    out_d[m * P:(m + 1) * P,
                                          c0 + g * 256:c0 + (g + 1) * 256],
                                    st[(m, 1)][:, so + g * 256:
                                       so + (g + 1) * 256])
                            continue
                        psum_t = ps.tile([P, 512], mybir.dt.float32,
                                         tag="psum", name=f"psum_{b}_{m}")
                        for k in range(KO):
                            nc.tensor.matmul(
                                psum_t[:], xt_ap(m, k), wB_ap(b, k),
                                start=(k == 0), stop=(k == KO - 1),
                            )
                        nc.scalar.copy(st[(m, 1)][:, so:so + 512], psum_t[:])
                        if b == 4:
                            nc.scalar.dma_start(
                                out_d[m * P:(m + 1) * P, HALF:N],
                                st[(m, 1)][:])
            else:
                # generic fallback (mt > 4): m-major over four 512-blocks
                for bi in range(4):
                    for m in range(mt):
                        psum_t = ps.tile([P, 512], mybir.dt.float32,
                                         tag="psum", name=f"ps_{bi}_{m}")
                        for k in range(KO):
                            if bi < 2:
                                rhs = wA_ap(k, bi)
                            else:
                                rhs = wB_ap(bi + 1, k)
                            nc.tensor.matmul(
                                psum_t[:], xt_ap(m, k), rhs,
                                start=(k == 0), stop=(k == KO - 1),
                            )
                        o_sb = st_p.tile([P, 512], OUT_DT,
                                         tag=f"o{(bi * mt + m) % 4}",
                                         name=f"o_{bi}_{m}")
                        nc.scalar.copy(o_sb[:], psum_t[:])
                        nc.scalar.dma_start(
                            out_d[m * P:(m + 1) * P,
                                  bi * 512:(bi + 1) * 512], o_sb[:])

    nc.compile()
    return nc


def _swizzle_x(x_pad: np.ndarray, t_pad: int) -> np.ndarray:
    # [t_pad, K] f32 -> [mt, P, KO, P] bf16, xt[mi,p,ko,j] = X[mi*P+j, ko*P+p]
    mt = t_pad // P
    v = x_pad.reshape(mt, P, KO, P).transpose(0, 3, 2, 1)
    return np.ascontiguousarray(v.astype(NP_COMPUTE))


def _swizzle_w(w_g: np.ndarray) -> np.ndarray:
    # [K, N] f32 -> [P, WTOT]: region A = cols 0:1024 k-major,
    # region B3 = cols 1024:1536, B4 = cols 1536:2048 (k-major each);
    # every DMA chunk is one contiguous run per partition line
    v = w_g.reshape(KO, P, N).transpose(1, 0, 2)  # [P, KO, N]
    parts = [
        np.ascontiguousarray(v[:, :, 0:HALF]).reshape(P, KO * HALF),
        np.ascontiguousarray(v[:, :, HALF:HALF + 512]).reshape(P, KO * 512),
        np.ascontiguousarray(v[:, :, HALF + 512:N]).reshape(P, KO * 512),
    ]
    return np.ascontiguousarray(np.concatenate(parts, axis=1).astype(NP_COMPUTE))


def _run(input, weight, tokens_per_expert, trace=False, **trace_kwargs):
    inp = np.ascontiguousarray(np.asarray(input), dtype=np.float32)
    wgt = np.ascontiguousarray(np.asarray(weight), dtype=np.float32)
    counts = np.asarray(tokens_per_expert).astype(np.int64)
    num_tokens, k = inp.shape
    assert k == K and wgt.shape == (G, K, N)
    # token group boundaries (matches searchsorted(cumsum, arange, 'right')),
    # clamped to the token range for safety on degenerate counts
    ends = np.minimum(np.cumsum(counts), num_tokens)
    starts = np.minimum(ends - counts, num_tokens)
    sizes = np.maximum(ends - starts, 0)

    t_pad = max(P, int(-(-max(int(sizes.max()), 1) // P)) * P)
    nc = _build(t_pad)

    in_maps = []
    for g in range(G):
        x_pad = np.zeros((t_pad, K), dtype=np.float32)
        x_pad[: sizes[g]] = inp[starts[g]:ends[g]]
        in_maps.append({"xt": _swizzle_x(x_pad, t_pad), "w": _swizzle_w(wgt[g])})

    res = bass_utils.run_bass_kernel_spmd(
        nc, in_maps, core_ids=list(range(G)), trace=trace, **trace_kwargs
    )

    # tokens not covered by any expert group get zero output (matches the
    # reference's masked accumulation)
    out = np.zeros((num_tokens, N), dtype=np.float32)
    for g in range(G):
        out[starts[g]:ends[g]] = res.results[g]["out"][: sizes[g]].astype(np.float32)
    return out, res


def kernel(input, weight, tokens_per_expert):
    out, _ = _run(input, weight, tokens_per_expert)
    return out
